# revision 11
# baseline (speedup 1.0000x reference)
"""Bass/Tile TRN2 kernel for nn_CA_66486093742236 (dense CA self-attention block).

Sharding: pure data parallel over batch (B=8 -> 8 cores, one batch element each).
Weights replicated to every core.

Per-core math (one batch element, x [256,4096], N=4096 spatial, C=64 channels):
  xf = convert_w @ x + convert_b                      [64, 4096]
  q  = q_w @ xf + q_b ; k = k_w @ xf + k_b            [64, 4096]
  S2[m,n] = sum_c k[c,m] q[c,n]   (= energy^T)        [4096, 4096], tiled
  E = exp(S2 - 2)  (global bias, cancels in the softmax ratio; keeps fp8 range)
  acc[c,n]  = sum_m vT0[m,c] E[m,n]   (vT0 = v^T without bias)
  den[n]    = sum_m E[m,n]   (ones column appended to vT0 -> row C of acc)
  gating: x0g = sigmoid(bn2(conv2_center @ relu(bn1(conv1_center @ mean_n(xf)))))
  out = (gamma/den[n])*acc[c,n] + (xf*(1+x0g) + gamma*v_b_eff)[c,n]

v3 design:
  - scalar-engine exp is the roofline (16.7M exps @ 1 elem/lane/cycle @1.2GHz
    ~= 110us + per-ACTIVATE overhead -> ~135us); everything else is scheduled
    to hide under it.
  - energy matmuls bf16 with row-half alternation: KQ=[k|q] and QK=[q|k]
    partition layouts let even m-blocks run as K=64 matmuls in PE rows 0-63
    and odd blocks in rows 64-127. Pairs are emitted back-to-back so the two
    matmuls run concurrently (row tiling) and LDWEIGHTS overlap.
  - AV matmuls fp8e4m3 + DoubleRow: one matmul per m-block pair (virtual-K
    packing); exp writes fp8 directly into a full-chunk SBUF ring
    [128, 32*512] so PSUM energy tiles are freed by the ACTIVATE itself.
  - flat pair-stream pipeline: all 128 (chunk, pair) energy steps form one
    stream; exp ACTIVATEs fire as soon as their 3-block group is covered;
    AV matmuls and chunk tails sit in a ready-queue drained with a 3-slot
    lag so the in-order PE queue NEVER waits on the scalar engine (v2 lost
    ~580ns/group to exactly that). Chunk j+1 pairs are pulled into the
    stage-A phase as soon as their k-chunks exist, keeping the scalar
    engine fed from ~12us on.
  - x is pre-cast to bf16 on the host: halves the input DMA (4MB->2MB) and
    the weight pack, and gives stage A separate (hideable) LDWEIGHTS.
  - gating sigmoid = 0.5*tanh(x/2)+0.5 (tanh shares exp's ACT table set ->
    no mid-kernel table reloads); its matmuls/ACTs are spread over the
    first four post-stage-A slots so they never head-of-line-block the PE.
"""

import os
import sys

sys.path.insert(0, "/opt/trn_rl_repo")

import heapq

import numpy as np

import concourse.bass as bass
import concourse.bacc as bacc
import concourse.tile as tile
from concourse import mybir
from concourse import library_config
from concourse.bass_utils import run_bass_kernel_spmd

F32 = mybir.dt.float32
BF16 = mybir.dt.bfloat16
F8 = mybir.dt.float8e4   # e4m3
AF = mybir.ActivationFunctionType
ALU = mybir.AluOpType
PM = mybir.MatmulPerfMode

B, CIN, C, H, W = 8, 256, 64, 64, 64
N = H * W                     # 4096
NCHUNK = 512                  # columns per n-chunk (one fp32 psum bank)
NCH = N // NCHUNK             # 8
MB = 128                      # m-block (energy partition block)
NMB = N // MB                 # 32 global m-blocks
MPC = NCHUNK // MB            # m-blocks per chunk (4)
NPAIR = NMB // 2              # 16 DoubleRow pairs per chunk
CP = C + 1                    # 65: attention acc rows + denominator row
CPAD = 80                     # padded vT channel stride (fp8 bytes, %16 == 0)
BN_RS = float(1.0 / np.sqrt(1.0 + 1e-5))
EXP_BIAS = -2.0               # exp(e-2): cancels in softmax ratio, fp8-safe

# bf16 [128, *] transposed-weight pack:
# wkqT0|wkqT1|wqkT0|wqkT1 (128 each) | cwT0|cwT1|vcwT0|vcwT1 (64 each)
WTRW = 4 * 128 + 4 * C
# fp32 [128, 3] bias pack: kqb | qkb | expbias
WPBW = 3
# fp32 [64, *] scalar pack: w1T|w2T (64 cols each) then one col each:
# cb, gv, rg, a1, b1, a2h, b2h
WSCW = 2 * C + 7

# m-blocks per exp group (3 psum banks per energy tile, double buffered = 6
# banks, leaving 2 banks for the attention accumulators)
M_GROUPS = [3] * 10 + [2]
assert sum(M_GROUPS) == NMB
NG = len(M_GROUPS)
GROUPS = []
_jm = 0
for _gs in M_GROUPS:
    GROUPS.append((_jm, _gs))
    _jm += _gs
AV_LAG = 3  # pair-slots between an exp ACTIVATE and the AV matmuls reading it

_last_results = None  # BassKernelResults of the most recent run (for test harness)


def _build_program():
    nc = bacc.Bacc("TRN2", target_bir_lowering=False, debug=False)

    x_d = nc.dram_tensor("x", [CIN, N], BF16, kind="ExternalInput").ap()
    wtr_d = nc.dram_tensor("wtr", [128, WTRW], BF16, kind="ExternalInput").ap()
    wpb_d = nc.dram_tensor("wpb", [128, WPBW], F32, kind="ExternalInput").ap()
    wsc_d = nc.dram_tensor("wsc", [C, WSCW], F32, kind="ExternalInput").ap()
    out_d = nc.dram_tensor("out", [C, N], F32, kind="ExternalOutput").ap()

    from contextlib import ExitStack

    with tile.TileContext(nc) as tc, ExitStack() as ctx:
        const = ctx.enter_context(tc.tile_pool(name="const", bufs=1))
        xinp = ctx.enter_context(tc.tile_pool(name="xinp", bufs=NCH))
        finp = ctx.enter_context(tc.tile_pool(name="finp", bufs=3))
        psum = ctx.enter_context(tc.tile_pool(name="psum", bufs=2, space="PSUM"))

        # ---------------- DMAs first (nothing queued ahead of them) ---------
        wtr = const.tile([128, WTRW], BF16)
        nc.sync.dma_start(out=wtr, in_=wtr_d)
        wpb = const.tile([128, WPBW], F32)
        nc.sync.dma_start(out=wpb, in_=wpb_d)
        wsc = const.tile([C, WSCW], F32)
        nc.sync.dma_start(out=wsc, in_=wsc_d)
        # x: one DMA per chunk, both 128-row halves in the free dim
        xt_t = []
        for j in range(NCH):
            xt = xinp.tile([128, 2, NCHUNK], BF16, tag="xin", name=f"xt{j}")
            nc.sync.dma_start(
                out=xt,
                in_=x_d[:, j * NCHUNK : (j + 1) * NCHUNK].rearrange(
                    "(h p) n -> p h n", h=2
                ),
            )
            xt_t.append(xt)

        # GPSIMD ucode library (only partition_broadcast, first used in the
        # chunk tails) -- loaded after the DMA triggers so its ~6us IRAM load
        # doesn't delay them
        nc.gpsimd.load_library(library_config.attn)

        wkqT0 = wtr[:, 0 * 128 : 1 * 128]
        wkqT1 = wtr[:, 1 * 128 : 2 * 128]
        wqkT0 = wtr[:, 2 * 128 : 3 * 128]
        wqkT1 = wtr[:, 3 * 128 : 4 * 128]
        _o = 4 * 128
        cwT0 = wtr[:, _o + 0 * C : _o + 1 * C]
        cwT1 = wtr[:, _o + 1 * C : _o + 2 * C]
        vcwT0 = wtr[:, _o + 2 * C : _o + 3 * C]
        vcwT1 = wtr[:, _o + 3 * C : _o + 4 * C]
        kqb_sb = wpb[:, 0:1]
        qkb_sb = wpb[:, 1:2]
        eb_sb = wpb[:, 2:3]

        w1T = wsc[:, 0:C]
        w2T = wsc[:, C : 2 * C]
        cb_sb = wsc[:, 2 * C + 0 : 2 * C + 1]
        gv_sb = wsc[:, 2 * C + 1 : 2 * C + 2]
        rg_sb = wsc[0:1, 2 * C + 2 : 2 * C + 3]
        a1_sb = wsc[:, 2 * C + 3 : 2 * C + 4]
        b1_sb = wsc[:, 2 * C + 4 : 2 * C + 5]
        a2h_sb = wsc[:, 2 * C + 5 : 2 * C + 6]
        b2h_sb = wsc[:, 2 * C + 6 : 2 * C + 7]

        # ---------------- persistent SBUF tiles ----------------
        # KQ[j]: k chunk j in partitions 0:64, q chunk j in partitions 64:128
        # QK[j]: q chunk j in partitions 0:64, k chunk j in partitions 64:128
        KQ_t = [const.tile([128, NCHUNK], BF16, name=f"KQ{j}") for j in range(NCH)]
        QK_t = [const.tile([128, NCHUNK], BF16, name=f"QK{j}") for j in range(NCH)]
        xf_t = [const.tile([C, NCHUNK], F32, name=f"xf{j}") for j in range(NCH)]
        # vT pairs: [ki, pair-in-chunk, ko, c] fp8; c stride padded to CPAD
        vT_t = [
            const.tile([128, 2, 2, CPAD], F8, name=f"vT{j}") for j in range(NCH)
        ]
        # exp ring: one full chunk of es (32 m-blocks x 512 n) in fp8
        esring = const.tile([128, NMB * NCHUNK], F8)

        for j in range(NCH):
            nc.vector.memset(vT_t[j][:, :, :, C : C + 1], 1.0)

        # ---------------- stage A ----------------
        def emit_stage_a_chunk(j):
            x0t = xt_t[j][:, 0, :]
            x1t = xt_t[j][:, 1, :]

            kqp = psum.tile([128, NCHUNK], F32, tag="eng", name=f"kqp{j}")
            nc.tensor.matmul(kqp, wkqT0, x0t, start=True, stop=False)
            nc.tensor.matmul(kqp, wkqT1, x1t, start=False, stop=True)
            nc.vector.tensor_scalar_add(KQ_t[j], kqp, kqb_sb)

            qkp = psum.tile([128, NCHUNK], F32, tag="eng", name=f"qkp{j}")
            nc.tensor.matmul(qkp, wqkT0, x0t, start=True, stop=False)
            nc.tensor.matmul(qkp, wqkT1, x1t, start=False, stop=True)
            nc.vector.tensor_scalar_add(QK_t[j], qkp, qkb_sb)

            xfp = psum.tile([C, NCHUNK], F32, tag="eng", name=f"xfp{j}")
            nc.tensor.matmul(xfp, cwT0, x0t, start=True, stop=False)
            nc.tensor.matmul(xfp, cwT1, x1t, start=False, stop=True)
            nc.vector.tensor_scalar_add(xf_t[j], xfp, cb_sb)

            # vT m-blocks of this chunk (no bias; v_b folded into final bias)
            vp = psum.tile([128, MPC * C], F32, tag="eng", name=f"vp{j}")
            for t in range(MPC):
                ms = slice(t * MB, (t + 1) * MB)
                nc.tensor.matmul(
                    vp[:, t * C : (t + 1) * C], x0t[:, ms], vcwT0,
                    start=True, stop=False,
                )
                nc.tensor.matmul(
                    vp[:, t * C : (t + 1) * C], x1t[:, ms], vcwT1,
                    start=False, stop=True,
                )
            nc.vector.tensor_copy(
                vT_t[j][:, :, :, 0:C],
                vp.rearrange("p (pr ko c) -> p pr ko c", pr=2, ko=2),
            )

        # ---------------- main pipeline state ----------------
        acc_t = [None] * NCH
        av_done = [0] * NCH      # pairs of AV matmuls emitted per chunk
        av_enq = [0] * NCH       # pairs enqueued per chunk
        acted = [0] * NCH        # exp groups emitted per chunk
        ep_tiles = {}
        ecnt = [0]               # global pair-slot counter
        tails_done = [0]
        gating_done = [False]
        from collections import deque

        avq = [deque() for _ in range(NCH)]  # (kind, ready, fn)

        def drain(force=0):
            # Emit ready AV/tail work. Only chunks [tails_done, tails_done+1]
            # may run AV matmuls (the acc psum ring holds two live
            # accumulators); a tail only runs once the gating chain exists
            # (fin2 reads the gating-updated xf).
            progress = True
            while progress:
                progress = False
                lo = tails_done[0]
                for j in range(lo, min(lo + 2, NCH)):
                    q = avq[j]
                    while q and (q[0][1] <= ecnt[0] or force > 0):
                        kind = q[0][0]
                        if kind == "tail" and not gating_done[0]:
                            break
                        if q[0][1] > ecnt[0]:
                            force -= 1
                        q.popleft()[2]()
                        progress = True
                    if tails_done[0] != lo:
                        break  # window moved; restart the scan

        def get_ep(j, g):
            key = (j, g)
            if key not in ep_tiles:
                ep_tiles[key] = psum.tile(
                    [128, 3 * NCHUNK], F32, tag="eng", name=f"ep{j}_{g}"
                )
            return ep_tiles[key]

        def emit_block_mm(j, mb):
            g = min(mb // 3, NG - 1)
            jm, _ = GROUPS[g]
            ep = get_ep(j, g)
            t = mb - jm
            jmc, sub = mb // MPC, mb % MPC
            msl = slice(sub * MB, (sub + 1) * MB)
            if mb % 2 == 0:
                lhsT = KQ_t[jmc][0:64, msl]       # k, rows 0-63
                rhs = QK_t[j][0:64, :]            # q, rows 0-63
            else:
                lhsT = QK_t[jmc][64:128, msl]     # k, rows 64-127
                rhs = KQ_t[j][64:128, :]          # q, rows 64-127
            nc.tensor.matmul(
                ep[:, t * NCHUNK : (t + 1) * NCHUNK], lhsT, rhs,
                start=True, stop=True,
            )

        def mk_av(j, p):
            def fn():
                if acc_t[j] is None:
                    acc_t[j] = psum.tile(
                        [CP, NCHUNK], F32, tag="acc", name=f"acc{j}"
                    )
                lhsT = vT_t[p // 2][:, p % 2, :, 0:CP]
                rhs = esring[
                    :, 2 * p * NCHUNK : (2 * p + 2) * NCHUNK
                ].rearrange("q (ko n) -> q ko n", ko=2)
                nc.tensor.matmul(
                    acc_t[j], lhsT, rhs,
                    perf_mode=PM.DoubleRow,
                    start=(p == 0), stop=(p == NPAIR - 1),
                )
                av_done[j] += 1
            return fn

        def mk_tail(j):
            def fn():
                acc = acc_t[j]
                # r = gamma/den (den = row C of acc; rg = 1/gamma host-side).
                # NOTE: custom-DVE ops mis-handle PSUM base_partition>0 on HW
                # -> copy the row to SBUF via a standard DVE op first.
                den_row = finp.tile([1, NCHUNK], F32, tag="den", name=f"den{j}")
                nc.vector.tensor_scalar_mul(den_row, acc[C : C + 1, :], rg_sb)
                r = finp.tile([1, NCHUNK], F32, tag="r", name=f"r{j}")
                nc.vector.reciprocal_approx_fast(r, den_row)
                rb_sb = finp.tile([C, NCHUNK], F32, tag="rb", name=f"rb{j}")
                nc.gpsimd.partition_broadcast(rb_sb, r)
                fin = finp.tile([C, NCHUNK], F32, tag="fin", name=f"fin{j}")
                nc.vector.tensor_mul(fin, acc[0:C, :], rb_sb)
                fin2 = finp.tile([C, NCHUNK], F32, tag="fin2", name=f"fin2{j}")
                nc.vector.tensor_add(fin2, fin, xf_t[j])
                nc.sync.dma_start(
                    out=out_d[:, j * NCHUNK : (j + 1) * NCHUNK], in_=fin2
                )
                tails_done[0] += 1
            return fn

        def emit_act(j, g):
            jm, gsize = GROUPS[g]
            ep = ep_tiles.pop((j, g))
            nc.scalar.activation(
                esring[:, jm * NCHUNK : (jm + gsize) * NCHUNK],
                ep[:, : gsize * NCHUNK],
                AF.Exp,
                bias=eb_sb,
            )
            ready = ecnt[0] + AV_LAG
            newp = (jm + gsize) // 2
            for p in range(av_enq[j], newp):
                avq[j].append(("av", ready, mk_av(j, p)))
            av_enq[j] = newp
            if g == NG - 1 and j > 0:
                avq[j].append(("tail", ready, mk_tail(j)))

        def emit_pair(j, P):
            drain()
            emit_block_mm(j, 2 * P)
            emit_block_mm(j, 2 * P + 1)
            ecnt[0] += 1
            while (
                acted[j] < NG
                and GROUPS[acted[j]][0] + GROUPS[acted[j]][1] <= 2 * P + 2
            ):
                emit_act(j, acted[j])
                acted[j] += 1

        # pair (j, P) needs: q(j) (stage A j), k-chunks of blocks 2P/2P+1,
        # and the previous chunk's AV matmuls over the same es-ring columns
        # (the exp ACTIVATE overwrites them; emission order = dependency
        # order in Tile, so the reader must be emitted first)
        def pair_eligible(j, P, jj):
            if j > jj or (2 * P + 1) // MPC > jj:
                return False
            if j > 0 and av_done[j - 1] < min(NPAIR, P + 2):
                return False
            return True

        nextP = [0] * NCH

        def pump(jj):
            # round-robin over chunks, skipping ineligible ones, so every
            # chunk whose k/q inputs exist feeds the scalar engine
            progress = True
            total = 0
            while progress:
                progress = False
                for j in range(NCH):
                    P = nextP[j]
                    if P < NPAIR and pair_eligible(j, P, jj):
                        nextP[j] += 1
                        emit_pair(j, P)
                        progress = True
                        total += 1
            return total

        # stage A with the pair pipeline riding along
        for jj in range(NCH):
            emit_stage_a_chunk(jj)
            pump(jj)

        # gating input: global mean of xf (DVE only)
        x0p = const.tile([C, NCH], F32)
        for j in range(NCH):
            nc.vector.tensor_reduce(
                x0p[:, j : j + 1], xf_t[j], axis=mybir.AxisListType.X, op=ALU.add
            )
        x0m = const.tile([C, 1], F32)
        nc.vector.tensor_reduce(x0m, x0p, axis=mybir.AxisListType.X, op=ALU.add)
        nc.vector.tensor_scalar_mul(x0m, x0m, 1.0 / N)
        y1s = const.tile([C, 1], F32)
        x0g = const.tile([C, 1], F32)
        fmul = const.tile([C, 1], F32)
        gate = {}

        def emit_gating_step(step):
            # psum tiles allocated at point of use so the eng-ring slot-reuse
            # dependencies line up with emission order
            if step == 0:
                gate["y1p"] = psum.tile([C, 1], F32, tag="eng", name="y1p")
                nc.tensor.matmul(gate["y1p"], w1T, x0m, start=True, stop=True)
            elif step == 1:
                nc.scalar.activation(
                    y1s, gate["y1p"], AF.Relu, bias=b1_sb, scale=a1_sb
                )
            elif step == 2:
                gate["y2p"] = psum.tile([C, 1], F32, tag="eng", name="y2p")
                nc.tensor.matmul(gate["y2p"], w2T, y1s, start=True, stop=True)
            elif step == 3:
                # sigmoid(z) = 0.5*tanh(z/2)+0.5; tanh shares exp's ACT table
                # set -> no table switches (a2/b2 pre-halved host-side)
                nc.scalar.activation(
                    x0g, gate["y2p"], AF.Tanh, bias=b2h_sb, scale=a2h_sb
                )
                nc.vector.tensor_scalar(
                    fmul, x0g, 0.5, 1.5, op0=ALU.mult, op1=ALU.add
                )
                # xf <- xf * (1.5+0.5*tanh) + gamma*v_b_eff (in place)
                for jj2 in range(NCH):
                    nc.vector.tensor_scalar(
                        xf_t[jj2], xf_t[jj2], fmul, gv_sb,
                        op0=ALU.mult, op1=ALU.add,
                    )
                # chunk 0's tail goes through the same queue; tails for j>0
                # were already enqueued and were held back by gating_done
                avq[0].append(("tail", ecnt[0] + 2, mk_tail(0)))
                gating_done[0] = True

        # drain the rest of the pair stream, gating steps on the first slots
        post = 0
        while any(nextP[j] < NPAIR for j in range(NCH)):
            if post <= 3:
                emit_gating_step(post)
            post += 1
            if pump(NCH - 1) == 0:
                assert any(avq), "pipeline stuck: guards with empty AV queues"
                drain(force=1)
        for s in range(min(post, 4), 4):
            emit_gating_step(s)
        while any(avq):
            drain(force=1)

    nc.compile()
    return nc


_program_cache = {}


def _get_program():
    if "p" not in _program_cache:
        _program_cache["p"] = _build_program()
    return _program_cache["p"]


def build_weight_inputs(inputs):
    import ml_dtypes

    def f64(v):
        return np.asarray(v, np.float64)

    cw = f64(inputs["convert_w"])        # [C, CIN]
    cb = f64(inputs["convert_b"])        # [C]
    qw, qb = f64(inputs["q_w"]), f64(inputs["q_b"])
    kw, kb = f64(inputs["k_w"]), f64(inputs["k_b"])
    vw, vb = f64(inputs["v_w"]), f64(inputs["v_b"])
    gamma = float(np.asarray(inputs["gamma"]).reshape(-1)[0])

    qcw = qw @ cw                        # [C, CIN]
    kcw = kw @ cw
    vcw = vw @ cw
    qbe = qw @ cb + qb                   # [C]
    kbe = kw @ cb + kb
    vbe = vw @ cb + vb

    wkq = np.concatenate([kcw, qcw], axis=0)   # [128, CIN]
    wqk = np.concatenate([qcw, kcw], axis=0)

    def tsplit(m):
        # [O, CIN] -> transposed halves [128, O] x2, bf16
        t = np.ascontiguousarray(m.T.astype(ml_dtypes.bfloat16))  # [CIN, O]
        return t[0:128], t[128:256]

    wkqT0, wkqT1 = tsplit(wkq)
    wqkT0, wqkT1 = tsplit(wqk)
    cwT0, cwT1 = tsplit(cw)
    vcwT0, vcwT1 = tsplit(vcw)
    wtr = np.concatenate(
        [wkqT0, wkqT1, wqkT0, wqkT1, cwT0, cwT1, vcwT0, vcwT1], axis=1
    )
    assert wtr.shape == (128, WTRW), wtr.shape

    kqbe = np.concatenate([kbe, qbe]).astype(np.float32)[:, None]  # [128,1]
    qkbe = np.concatenate([qbe, kbe]).astype(np.float32)[:, None]
    wpb = np.concatenate(
        [kqbe, qkbe, np.full((128, 1), EXP_BIAS, np.float32)], axis=1
    )
    assert wpb.shape == (128, WPBW), wpb.shape

    w1c = f64(inputs["conv1_w"]).reshape(C, C, 3, 3)[:, :, 1, 1]
    w2c = f64(inputs["conv2_w"]).reshape(C, C, 3, 3)[:, :, 1, 1]
    a1 = f64(inputs["bn1_g"]) * BN_RS
    b1f = a1 * f64(inputs["conv1_b"]) + f64(inputs["bn1_b"])
    a2 = f64(inputs["bn2_g"]) * BN_RS
    b2f = a2 * f64(inputs["conv2_b"]) + f64(inputs["bn2_b"])

    cols = [
        w1c.T.astype(np.float32),
        w2c.T.astype(np.float32),
        cb.astype(np.float32)[:, None],
        (gamma * vbe).astype(np.float32)[:, None],
        np.full((C, 1), 1.0 / gamma, np.float32),
        a1.astype(np.float32)[:, None],
        b1f.astype(np.float32)[:, None],
        (a2 / 2).astype(np.float32)[:, None],
        (b2f / 2).astype(np.float32)[:, None],
    ]
    wsc = np.concatenate(cols, axis=1)
    assert wsc.shape == (C, WSCW), wsc.shape

    return {
        "wtr": np.ascontiguousarray(wtr),
        "wpb": np.ascontiguousarray(wpb),
        "wsc": np.ascontiguousarray(wsc),
    }


def kernel(**inputs: np.ndarray) -> np.ndarray:
    global _last_results
    import ml_dtypes

    x = np.ascontiguousarray(np.asarray(inputs["x"], dtype=np.float32))
    assert x.shape == (B, CIN, H, W)
    weights = build_weight_inputs(inputs)
    nc = _get_program()

    in_maps = []
    for b in range(B):
        m = dict(weights)
        m["x"] = np.ascontiguousarray(
            x[b].reshape(CIN, N).astype(ml_dtypes.bfloat16)
        )
        in_maps.append(m)

    trace = bool(int(os.environ.get("KERNEL_TRACE", "0")))
    res = run_bass_kernel_spmd(nc, in_maps, list(range(B)), trace=trace)
    _last_results = res

    out = np.stack([res.results[b]["out"].reshape(C, H, W) for b in range(B)], axis=0)
    return out.astype(np.float32)
